# revision 4
# baseline (speedup 1.0000x reference)
"""Trainium2 Bass kernel for the 6-level hierarchical Choquet integral tree.

Tree: 16-ary, depth 6, 16.7M leaves. Each node: softmax(theta) over 136
coeffs (16 singles + 120 pair-mins), dot with [children ; pairwise mins].

v3 design:
- Host precomputes m = softmax(theta) in f32 (theta-only -> static), ships
  bf16, feature-major per tile row: col = f*16 + g (g = node-in-row).
- Pair mins via wrapped rotations: d=1..7 pairs (i, (i+d)%16) i=0..15, d=8
  i=0..7 -> all 120 unordered pairs once. xs ships duplicated ([xs|xs]) so
  every rotation is contiguous; d=1..7 fuse into ONE tensor_tensor via an
  overlapping-window AP (stride-16 window dim) + stride-0 broadcast src.
- Dot via 2 contiguous mults + tree over feature rows (136->68->34->17) +
  one strided grouped reduce -> [p, 16] f32. Tree step 1 runs on the Pool
  (GpSimd) engine to offload the DVE.
- Two 2048-node tiles fused per instruction pass (halves dispatch
  overhead); levels 1-2 on device, levels 3-6 on host (4369 nodes, numpy).
"""

import os

import numpy as np

import bass_rust
import concourse.bass as bass
import concourse.mybir as mybir
import concourse.tile as tile
from concourse import bacc
from concourse.bass_utils import run_bass_kernel_spmd

B = 16
NPAIR = 120
NF = B + NPAIR  # 136
W = B * NF      # 2176 cols per tile row
NCORE = 8
LEAF_PER_CORE = 16**6 // NCORE  # 2,097,152
N1 = LEAF_PER_CORE // B         # 131072 level-1 nodes/core
N2 = N1 // B                    # 8192  level-2 nodes/core
T1 = 64                         # level-1 tiles of 2048 nodes (128p x 16)
J1 = T1 // 2                    # fused level-1 passes
Q2 = 4                          # level-2 sub-tiles of 2048 nodes
J2 = Q2 // 2                    # fused level-2 passes

_F32 = mybir.dt.float32
_BF = mybir.dt.bfloat16


def _pair_perm() -> np.ndarray:
    """Wrapped-rotation pair position q -> natural pair index (0..119)."""
    II, JJ = np.triu_indices(B, k=1)
    nat = {(int(a), int(b)): p for p, (a, b) in enumerate(zip(II, JJ))}
    perm = []
    for d in range(1, 9):
        for i in range(B if d < 8 else 8):
            j = (i + d) % B
            perm.append(nat[(min(i, j), max(i, j))])
    assert len(perm) == NPAIR and len(set(perm)) == NPAIR
    return np.array(perm, dtype=np.int64)


PAIR_PERM = _pair_perm()


def _win_ap(ap, t_stride, d0, nd):
    """4D overlapping-window view [128, 2, nd, 256] of a [128, >=1024]
    tile AP: dims (tile-pair stride t_stride, window stride 16 starting at
    col d0*16, run 256). Window stride 0 = broadcast."""
    a = ap[:, d0 * B : d0 * B + 256].unsqueeze(1).unsqueeze(1)
    a = a.broadcast_to([128, 2, nd, 256])
    raw = list(a.ap)
    raw[1] = (t_stride, 2)
    raw[2] = (raw[2][0], nd)
    a.ap = bass_rust.VecI64Pair(raw)
    return a


def _win_ap_d(ap, t_stride, nd):
    a = _win_ap(ap, t_stride, 1, nd)
    raw = list(a.ap)
    raw[2] = (B, nd)
    a.ap = bass_rust.VecI64Pair(raw)
    return a


def _kernel_pass(nc, pools, m_src, xs_src, xs2_ap, out_cb):
    """One fused pass over two 2048-node tiles.

    m_src:  DRAM AP [128, 2*2176] bf16 (feature-major softmax weights).
    xs_src: DRAM AP [128, 2*512] or None (xs2_ap pre-filled by caller).
    xs2_ap: SBUF AP [128, 1024]: per tile [xs(256)|xs(256)] f-major.
    out_cb(dot_ap): consume the [128, 32] f32 result ([t*16+g])."""
    mp, minp, pp, tp, sp = pools

    m_t = mp.tile([128, 2 * W], _BF, tag="m")
    nc.sync.dma_start(out=m_t[:], in_=m_src)
    if xs_src is not None:
        nc.sync.dma_start(out=xs2_ap, in_=xs_src)

    # pair mins: d=1..7 in one op (overlapping windows), d=8 separate
    mn_t = minp.tile([128, 2 * 1920], _BF, tag="mn")
    mn3 = mn_t[:].rearrange("p (t f) -> p t f", t=2)
    nc.vector.tensor_tensor(
        mn3[:, :, 0:1792].rearrange("p t (d i) -> p t d i", d=7),
        _win_ap(xs2_ap, 512, 0, 7),
        _win_ap_d(xs2_ap, 512, 7),
        op=mybir.AluOpType.min,
    )
    xs3 = xs2_ap.rearrange("p (t f) -> p t f", t=2)
    nc.vector.tensor_tensor(
        mn3[:, :, 1792:1920],
        xs3[:, :, 0:128],
        xs3[:, :, 128:256],
        op=mybir.AluOpType.min,
    )

    # products: P[t, 0:256] = m_s * xs, P[t, 256:2176] = m_p * mins
    p_t = pp.tile([128, 2 * W], _BF, tag="p")
    p3 = p_t[:].rearrange("p (t f) -> p t f", t=2)
    m3 = m_t[:].rearrange("p (t f) -> p t f", t=2)
    nc.vector.tensor_tensor(
        p3[:, :, 0:256], m3[:, :, 0:256], xs3[:, :, 0:256],
        op=mybir.AluOpType.mult,
    )
    nc.vector.tensor_tensor(
        p3[:, :, 256:], m3[:, :, 256:], mn3[:], op=mybir.AluOpType.mult,
    )

    # tree: 136 -> 68 (Pool engine) -> 34 -> 17 -> grouped reduce
    t1 = tp.tile([128, 2 * 1088], _BF, tag="t1")
    t13 = t1[:].rearrange("p (t f) -> p t f", t=2)
    nc.gpsimd.tensor_tensor(
        t13[:], p3[:, :, 0:1088], p3[:, :, 1088:2176],
        op=mybir.AluOpType.add,
    )
    t2 = tp.tile([128, 2 * 544], _BF, tag="t2")
    t23 = t2[:].rearrange("p (t f) -> p t f", t=2)
    nc.vector.tensor_tensor(
        t23[:], t13[:, :, 0:544], t13[:, :, 544:1088],
        op=mybir.AluOpType.add,
    )
    t3 = tp.tile([128, 2 * 272], _BF, tag="t3")
    t33 = t3[:].rearrange("p (t f) -> p t f", t=2)
    nc.vector.tensor_tensor(
        t33[:], t23[:, :, 0:272], t23[:, :, 272:544],
        op=mybir.AluOpType.add,
    )
    dot = sp.tile([128, 32], _F32, tag="dot")
    nc.vector.tensor_reduce(
        dot[:],
        t3[:].rearrange("p (t r g) -> p t g r", t=2, r=17),
        axis=mybir.AxisListType.X,
        op=mybir.AluOpType.add,
    )
    out_cb(dot)


def _build_program() -> bass.Bass:
    nc = bacc.Bacc("TRN2", target_bir_lowering=False, debug=False)

    m1_d = nc.dram_tensor("m1", [J1 * 128 * 2 * W], _BF, kind="ExternalInput")
    x_d = nc.dram_tensor("xd", [J1 * 128 * 1024], _BF, kind="ExternalInput")
    m2_d = nc.dram_tensor("m2", [J2 * 128 * 2 * W], _BF, kind="ExternalInput")
    o2_d = nc.dram_tensor("o2", [128 * 64], _F32, kind="ExternalOutput")

    m1_src = m1_d.ap().rearrange("(j p f) -> j p f", p=128, f=2 * W)
    x_src = x_d.ap().rearrange("(j p f) -> j p f", p=128, f=1024)
    m2_src = m2_d.ap().rearrange("(j p f) -> j p f", p=128, f=2 * W)
    o2_dst = o2_d.ap().rearrange("(p t) -> p t", t=64)

    with tile.TileContext(nc) as tc:
        with (
            tc.tile_pool(name="m", bufs=3) as mp,
            tc.tile_pool(name="xs", bufs=3) as xsp,
            tc.tile_pool(name="mn", bufs=2) as minp,
            tc.tile_pool(name="pr", bufs=2) as pp,
            tc.tile_pool(name="tr", bufs=2) as tp,
            tc.tile_pool(name="sm", bufs=4) as sp,
            tc.tile_pool(name="v1", bufs=1) as v1p,
        ):
            pools = (mp, minp, pp, tp, sp)
            # level-1 -> level-2 staging: per sub-tile q: [xs(256)|xs(256)]
            v1buf = v1p.tile([128, Q2 * 512], _BF, tag="v1buf")

            def mk_store(j):
                def store(dot):
                    for h2 in range(2):  # tile halves
                        t = 2 * j + h2
                        q, tl = t // 16, t % 16
                        blk = v1buf[:, q * 512 : (q + 1) * 512].rearrange(
                            "p (h i g) -> p h i g", h=2, i=B)
                        src = dot[:, h2 * B : (h2 + 1) * B].rearrange(
                            "p (i o) -> p i o", o=1)
                        for h in (0, 1):
                            nc.scalar.activation(
                                blk[:, h, :, tl : tl + 1], src,
                                mybir.ActivationFunctionType.Copy,
                            )
                return store

            for j in range(J1):
                xs2_t = xsp.tile([128, 1024], _BF, tag="xs2")
                _kernel_pass(nc, pools, m1_src[j], x_src[j], xs2_t[:],
                             mk_store(j))

            for j in range(J2):
                def store2(dot, j=j):
                    nc.sync.dma_start(
                        out=o2_dst[:, j * 32 : (j + 1) * 32], in_=dot[:])
                _kernel_pass(nc, pools, m2_src[j], None,
                             v1buf[:, j * 1024 : (j + 1) * 1024], store2)

    nc.compile()
    return nc


def _choquet_np(vals: np.ndarray, theta: np.ndarray) -> np.ndarray:
    II, JJ = np.triu_indices(B, k=1)
    n = theta.shape[0]
    xs = vals.reshape(n, B).astype(np.float64)
    t = theta.astype(np.float64)
    e = np.exp(t - t.max(axis=1, keepdims=True))
    m = e / e.sum(axis=1, keepdims=True)
    mins = np.minimum(xs[:, II], xs[:, JJ])
    return (m[:, :B] * xs).sum(axis=1) + (m[:, B:] * mins).sum(axis=1)


def _softmax_f32(theta: np.ndarray) -> np.ndarray:
    t = np.asarray(theta, dtype=np.float32)
    e = np.exp(t - t.max(axis=1, keepdims=True))
    return e / e.sum(axis=1, keepdims=True)


_PROG_CACHE: bass.Bass | None = None
LAST_RESULTS = None


def _ensure_ntff_hook() -> None:
    """Provide antenv.axon_hooks + the ctypes NTFF hook when the image
    lacks them, so trace=True produces a perfetto profile under axon."""
    import contextlib
    import ctypes
    import sys
    import types

    try:
        from antenv.axon_hooks import get_axon_ntff_profile_hook  # noqa: F401

        return
    except ImportError:
        pass

    import antenv
    import concourse.bass_utils as bu

    holder = {"h": None}
    mod = types.ModuleType("antenv.axon_hooks")
    mod.set_axon_ntff_profile_hook = lambda h: holder.__setitem__("h", h)
    mod.get_axon_ntff_profile_hook = lambda: holder["h"]
    sys.modules["antenv.axon_hooks"] = mod
    antenv.axon_hooks = mod
    bu.upload_artifacts = lambda tmpdir: ""

    so_path = "/opt/axon/libaxon_pjrt.so"
    try:
        lib = ctypes.CDLL(so_path)
    except OSError:
        return
    if not hasattr(lib, "axon_start_nrt_profile"):
        return
    lib.axon_start_nrt_profile.argtypes = [
        ctypes.POINTER(ctypes.c_int64),
        ctypes.c_size_t,
    ]
    lib.axon_start_nrt_profile.restype = ctypes.c_int64
    lib.axon_stop_nrt_profile.argtypes = [ctypes.c_char_p]
    lib.axon_stop_nrt_profile.restype = ctypes.c_int64

    @contextlib.contextmanager
    def _hook(output_dir, device_ids):
        import jax

        jax.devices()
        if device_ids:
            ids = (ctypes.c_int64 * len(device_ids))(*device_ids)
            rc = lib.axon_start_nrt_profile(ids, len(device_ids))
        else:
            rc = lib.axon_start_nrt_profile(None, 0)
        if rc != 0:
            raise RuntimeError(f"axon_start_nrt_profile rc={rc}")
        try:
            yield
        finally:
            n = lib.axon_stop_nrt_profile(str(output_dir).encode())
            print(f"profile: {n} file(s) written to {output_dir}")

    mod.set_axon_ntff_profile_hook(_hook)


def kernel(x, theta1, theta2, theta3, theta4, theta5, theta6) -> np.ndarray:
    global _PROG_CACHE, LAST_RESULTS
    import ml_dtypes

    x = np.ascontiguousarray(np.asarray(x, dtype=np.float32).reshape(-1))
    m1 = _softmax_f32(np.asarray(theta1, dtype=np.float32))
    m2 = _softmax_f32(np.asarray(theta2, dtype=np.float32))
    cols = np.concatenate([np.arange(B), B + PAIR_PERM])

    if _PROG_CACHE is None:
        _PROG_CACHE = _build_program()
    nc = _PROG_CACHE

    in_maps = []
    for c in range(NCORE):
        xc = x[c * LEAF_PER_CORE : (c + 1) * LEAF_PER_CORE]
        # leaf n = t*32768 + p*256 + g*16 + i -> xs[t, p, i*16+g], dup'd
        xs = xc.reshape(T1, 128, B, B).transpose(0, 1, 3, 2)  # (t,p,i,g)
        xs = xs.reshape(T1, 128, 256).astype(ml_dtypes.bfloat16)
        xd = np.concatenate([xs, xs], axis=2)                 # (t,p,512)
        xd = xd.reshape(J1, 2, 128, 512).transpose(0, 2, 1, 3)  # (j,p,t,512)

        # level-1 weights: node n = t*2048 + p*16 + g -> (t, p, f*16+g)
        m1c = m1[c * N1 : (c + 1) * N1][:, cols]
        m1c = m1c.reshape(T1, 128, B, NF).transpose(0, 1, 3, 2)  # (t,p,f,g)
        m1c = m1c.reshape(J1, 2, 128, W).transpose(0, 2, 1, 3)   # (j,p,t,W)

        # level-2: node j2 = t*128 + p -> sub-tile q=t//16, col f*16+(t%16)
        m2c = m2[c * N2 : (c + 1) * N2][:, cols]
        m2c = m2c.reshape(64, 128, NF).transpose(1, 0, 2)        # (p, t, f)
        m2c = m2c.reshape(128, Q2, B, NF).transpose(1, 0, 3, 2)  # (q,p,f,t)
        m2c = m2c.reshape(J2, 2, 128, W).transpose(0, 2, 1, 3)   # (j,p,q,W)

        in_maps.append({
            "m1": np.ascontiguousarray(
                m1c.astype(ml_dtypes.bfloat16)).reshape(-1),
            "xd": np.ascontiguousarray(xd).reshape(-1),
            "m2": np.ascontiguousarray(
                m2c.astype(ml_dtypes.bfloat16)).reshape(-1),
        })

    trace = os.environ.get("BASS_KERNEL_TRACE", "0") == "1"
    if trace:
        _ensure_ntff_hook()
    res = run_bass_kernel_spmd(nc, in_maps, list(range(NCORE)), trace=trace)
    LAST_RESULTS = res

    # o2[p, t] = level-2 node j2 = t*128 + p
    l2 = np.concatenate([
        np.asarray(res.results[c]["o2"], dtype=np.float32)
        .reshape(128, 64).T.reshape(-1)
        for c in range(NCORE)
    ])
    vals = l2
    for th in (theta3, theta4, theta5, theta6):
        vals = _choquet_np(vals, np.asarray(th, dtype=np.float32))
    return vals.astype(np.float32).reshape((1,))
